# revision 3
# baseline (speedup 1.0000x reference)
"""Multi-head attention (B=4, S=2048, D=1024, H=16) on 8 trn2 NeuronCores.

Sharding: batch (4-way) x head-half (2-way).  Core c = 2*b + hh handles batch b
and heads hh*8 .. hh*8+7.  Each core:
  1. QT/KT projections in d-on-partitions layout, V in tokens-on-partitions
     layout with a ones-column per head (softmax denominator comes out of the
     attn@V matmul as row 64).  All matmuls run as float32r (full PE rate,
     fp32-matmul numerics).
  2. Per (head, key-tile): scores_T = K_h @ Q_h^T into a 4-bank PSUM tile, one
     wide exp on the scalar engine -> attn_T slice (streamed flash-style, no
     SxS materialization), attn@V accumulates out_T per head in PSUM.
  3. out_T / den (denominator row replicated across partitions with two
     32-lane stream shuffles), then the output projection consumes out_T
     directly as lhsT.  Host sums the two half-head partials and adds bias.
"""

import sys

if "/opt/trn_rl_repo" not in sys.path:
    sys.path.insert(0, "/opt/trn_rl_repo")

import numpy as np

B, S, D = 4, 2048, 1024
H, HD = 16, 64
P = 128
DK = D // P            # 8 contraction chunks for the projections
NKT = S // P           # 16 token tiles
QB = 512
NQB = S // QB          # 4 q blocks per matmul row
DH = 512               # head dims per core (8 heads)
NDC = DH // P          # 4 dout chunks per core
NHC = 8                # heads per core
VW = HD + 1            # V columns per head incl. the ones column
NCORES = 8

_PROG = [None]


def _build():
    import concourse.mybir as mybir
    import concourse.tile as tile
    from concourse import bacc

    f32 = mybir.dt.float32
    f32r = mybir.dt.float32r
    Exp = mybir.ActivationFunctionType.Exp

    nc = bacc.Bacc("TRN2", target_bir_lowering=False, debug=False)
    xq = nc.dram_tensor("xq", [D, S], f32r, kind="ExternalInput").ap()
    xk = nc.dram_tensor("xk", [D, S], f32r, kind="ExternalInput").ap()
    xv = nc.dram_tensor("xv", [D, S], f32r, kind="ExternalInput").ap()
    wq = nc.dram_tensor("wq", [D, DH], f32r, kind="ExternalInput").ap()
    wk = nc.dram_tensor("wk", [D, DH], f32r, kind="ExternalInput").ap()
    wv = nc.dram_tensor("wv", [D, DH], f32r, kind="ExternalInput").ap()
    wo = nc.dram_tensor("wo", [DH, D], f32r, kind="ExternalInput").ap()
    part = nc.dram_tensor("part", [S, D], f32, kind="ExternalOutput").ap()

    xq_v = xq.rearrange("(c p) s -> p c s", p=P)
    xk_v = xk.rearrange("(c p) s -> p c s", p=P)
    xv_v = xv.rearrange("(c p) s -> p c s", p=P)

    with tile.TileContext(nc) as tc:
        with (
            tc.tile_pool(name="big", bufs=1) as big,
            tc.tile_pool(name="ps", bufs=2, space="PSUM") as psum,
        ):
            QT = big.tile([P, NDC, S], f32r, tag="QT")
            KT = big.tile([P, NDC, S], f32r, tag="KT")
            V = big.tile([P, NKT, NHC * VW], f32r, tag="V")
            outT = big.tile([P, NDC, S], f32r, tag="outT")

            # ---- projections ------------------------------------------------
            with tc.tile_pool(name="xc", bufs=10) as xc:

                def proj_T(x_view, w_dram, out_t):
                    w_t = big.tile([P, DK, DH], f32r, tag="w")
                    nc.sync.dma_start(
                        w_t[:], w_dram.rearrange("(c p) m -> p c m", p=P)
                    )
                    for qb in range(NQB):
                        xts = []
                        for dk in range(DK):
                            xt = xc.tile([P, QB], f32r, tag="xc")
                            nc.sync.dma_start(
                                xt[:], x_view[:, dk, qb * QB : (qb + 1) * QB]
                            )
                            xts.append(xt)
                        for dc in range(NDC):
                            pt = psum.tile([P, QB], f32, tag="ps")
                            for dk in range(DK):
                                nc.tensor.matmul(
                                    pt[:],
                                    w_t[:, dk, dc * P : (dc + 1) * P],
                                    xts[dk][:],
                                    start=(dk == 0),
                                    stop=(dk == DK - 1),
                                )
                            nc.vector.tensor_copy(
                                out_t[:, dc, qb * QB : (qb + 1) * QB], pt[:]
                            )

                proj_T(xq_v, wq, QT)
                proj_T(xk_v, wk, KT)

                # V projection (tokens-on-partitions) + ones columns
                nc.vector.memset(V[:].bitcast(f32), 1.0)
                wv_t = big.tile([P, DK, DH], f32r, tag="w")
                nc.sync.dma_start(wv_t[:], wv.rearrange("(c p) m -> p c m", p=P))
                for qb in range(NQB):
                    xts = []
                    for dk in range(DK):
                        xt = xc.tile([P, QB], f32r, tag="xc")
                        nc.sync.dma_start(
                            xt[:], xv_v[:, dk, qb * QB : (qb + 1) * QB]
                        )
                        xts.append(xt)
                    for kt_in in range(QB // P):
                        kt = qb * (QB // P) + kt_in
                        pt = psum.tile([P, DH], f32, tag="ps")
                        for dk in range(DK):
                            nc.tensor.matmul(
                                pt[:],
                                xts[dk][:, kt_in * P : (kt_in + 1) * P],
                                wv_t[:, dk, :],
                                start=(dk == 0),
                                stop=(dk == DK - 1),
                            )
                        nc.vector.tensor_copy(
                            V[:, kt].rearrange("p (h c) -> p h c", c=VW)[
                                :, :, 0:HD
                            ],
                            pt[:].rearrange("p (h c) -> p h c", c=HD),
                        )

            # wo loads while attention runs (slot frees when wv_t is done)
            wo_t = big.tile([P, NDC, D], f32r, tag="w")
            nc.sync.dma_start(wo_t[:], wo.rearrange("(c p) m -> p c m", p=P))

            # ---- attention --------------------------------------------------
            with tc.tile_pool(name="attn", bufs=2) as attnp, tc.tile_pool(
                name="small", bufs=1
            ) as small:
                for h in range(NHC):
                    hp, hc = h % 2, h // 2
                    r0 = 64 * hp
                    acc = psum.tile([P, S], f32, tag="ps")
                    for kt in range(NKT):
                        sc = psum.tile([P, S], f32, tag="ps")
                        for qb in range(NQB):
                            nc.tensor.matmul(
                                sc[:, qb * QB : (qb + 1) * QB],
                                KT[r0 : r0 + 64, hc, kt * P : (kt + 1) * P],
                                QT[r0 : r0 + 64, hc, qb * QB : (qb + 1) * QB],
                                start=True,
                                stop=True,
                            )
                        at_t = attnp.tile([P, S], f32r, tag="attn")
                        nc.scalar.activation(at_t[:], sc[:], Exp)
                        for qb in range(NQB):
                            nc.tensor.matmul(
                                acc[0:VW, qb * QB : (qb + 1) * QB],
                                V[:, kt, h * VW : (h + 1) * VW],
                                at_t[:, qb * QB : (qb + 1) * QB],
                                start=(kt == 0),
                                stop=(kt == NKT - 1),
                            )
                    # denominator (acc row 64) -> all 64 partitions
                    bc = small.tile([64, S], f32, tag="bc")
                    nc.vector.stream_shuffle(
                        bc[0:32, :], acc[64:96, :], [0] * 32
                    )
                    nc.vector.stream_shuffle(
                        bc[32:64, :], acc[64:96, :], [0] * 32
                    )
                    rec = small.tile([64, S], f32, tag="rec")
                    nc.vector.reciprocal(rec[:], bc[:])
                    if hp == 0:
                        nc.vector.tensor_mul(
                            outT[0:64, hc, :], acc[0:HD, :], rec[:]
                        )
                    else:
                        tmp = small.tile([64, S], f32r, tag="tmp")
                        nc.vector.tensor_mul(tmp[:], acc[0:HD, :], rec[:])
                        nc.sync.dma_start(outT[64:128, hc, :], tmp[:])

            # ---- output projection -----------------------------------------
            with tc.tile_pool(name="stage", bufs=2) as stage:
                for qt in range(NKT):
                    po = psum.tile([P, D], f32, tag="ps")
                    for do in range(2):
                        for dc in range(NDC):
                            nc.tensor.matmul(
                                po[:, do * QB : (do + 1) * QB],
                                outT[:, dc, qt * P : (qt + 1) * P],
                                wo_t[:, dc, do * QB : (do + 1) * QB],
                                start=(dc == 0),
                                stop=(dc == NDC - 1),
                            )
                    st = stage.tile([P, D], f32, tag="st")
                    nc.vector.tensor_copy(st[:], po[:])
                    nc.sync.dma_start(part[qt * P : (qt + 1) * P, :], st[:])

    nc.compile()
    return nc


def _get_prog():
    if _PROG[0] is None:
        _PROG[0] = _build()
    return _PROG[0]


def make_in_maps(query, key, value, Wq, Wk, Wv, Wo):
    scale = np.float32(1.0 / np.sqrt(D))
    Wq_s = (np.asarray(Wq, np.float32) * scale).astype(np.float32)
    Wk_s = np.ascontiguousarray(np.asarray(Wk, np.float32))
    Wv_s = np.ascontiguousarray(np.asarray(Wv, np.float32))
    Wo_s = np.ascontiguousarray(np.asarray(Wo, np.float32))
    in_maps = []
    for b in range(B):
        xqT = np.ascontiguousarray(np.asarray(query[b], np.float32).T)
        xkT = np.ascontiguousarray(np.asarray(key[b], np.float32).T)
        xvT = np.ascontiguousarray(np.asarray(value[b], np.float32).T)
        for hh in range(2):
            sl = slice(hh * DH, (hh + 1) * DH)
            in_maps.append(
                {
                    "xq": xqT,
                    "xk": xkT,
                    "xv": xvT,
                    "wq": np.ascontiguousarray(Wq_s[:, sl]),
                    "wk": np.ascontiguousarray(Wk_s[:, sl]),
                    "wv": np.ascontiguousarray(Wv_s[:, sl]),
                    "wo": np.ascontiguousarray(Wo_s[sl, :]),
                }
            )
    return in_maps


def run(in_maps, trace=False, **kw):
    from concourse.bass_utils import run_bass_kernel_spmd

    nc = _get_prog()
    return run_bass_kernel_spmd(
        nc, in_maps, core_ids=list(range(NCORES)), trace=trace, **kw
    )


def kernel(query, key, value, Wq, Wk, Wv, Wo, bo):
    in_maps = make_in_maps(query, key, value, Wq, Wk, Wv, Wo)
    res = run(in_maps)
    bo = np.asarray(bo, np.float32)
    out = np.empty((B, S, D), np.float32)
    for b in range(B):
        out[b] = res.results[2 * b]["part"] + res.results[2 * b + 1]["part"] + bo
    return out


# revision 4
# speedup vs baseline: 1.5357x; 1.5357x over previous
"""Multi-head attention (B=4, S=2048, D=1024, H=16) on 8 trn2 NeuronCores.

Sharding: batch (4-way) x head-half (2-way).  Core c = 2*b + hh handles batch b
and heads hh*8 .. hh*8+7.  Each core:
  1. QT/KT projections in d-on-partitions layout, V in tokens-on-partitions
     layout with a ones-column per head (softmax denominator comes out of the
     attn@V matmul as row 64).  All matmuls run as float32r (full PE rate,
     fp32-matmul numerics).
  2. Per (head, key-tile): scores_T = K_h @ Q_h^T into a 4-bank PSUM tile, one
     wide exp on the scalar engine -> attn_T slice (streamed flash-style, no
     SxS materialization), attn@V accumulates out_T per head in PSUM.
  3. out_T / den (denominator row replicated across partitions with two
     32-lane stream shuffles), then the output projection consumes out_T
     directly as lhsT.  Host sums the two half-head partials and adds bias.
"""

import sys

if "/opt/trn_rl_repo" not in sys.path:
    sys.path.insert(0, "/opt/trn_rl_repo")

import numpy as np

B, S, D = 4, 2048, 1024
H, HD = 16, 64
P = 128
DK = D // P            # 8 contraction chunks for the projections
NKT = S // P           # 16 token tiles
QB = 512
NQB = S // QB          # 4 q blocks per matmul row
DH = 512               # head dims per core (8 heads)
NDC = DH // P          # 4 dout chunks per core
NHC = 8                # heads per core
VW = HD + 1            # V columns per head incl. the ones column
NCORES = 8

_PROG = [None]


def _build():
    import concourse.mybir as mybir
    import concourse.tile as tile
    from concourse import bacc

    f32 = mybir.dt.float32
    f32r = mybir.dt.float32r
    Exp = mybir.ActivationFunctionType.Exp

    nc = bacc.Bacc("TRN2", target_bir_lowering=False, debug=False)
    xq = nc.dram_tensor("xq", [D, S], f32r, kind="ExternalInput").ap()
    xk = nc.dram_tensor("xk", [D, S], f32r, kind="ExternalInput").ap()
    xv = nc.dram_tensor("xv", [D, S], f32r, kind="ExternalInput").ap()
    wq = nc.dram_tensor("wq", [D, DH], f32r, kind="ExternalInput").ap()
    wk = nc.dram_tensor("wk", [D, DH], f32r, kind="ExternalInput").ap()
    wv = nc.dram_tensor("wv", [D, DH], f32r, kind="ExternalInput").ap()
    wo = nc.dram_tensor("wo", [DH, D], f32r, kind="ExternalInput").ap()
    part = nc.dram_tensor("part", [S, D], f32, kind="ExternalOutput").ap()

    xq_v = xq.rearrange("(c p) s -> p c s", p=P)
    xk_v = xk.rearrange("(c p) s -> p c s", p=P)
    xv_v = xv.rearrange("(c p) s -> p c s", p=P)

    with tile.TileContext(nc) as tc:
        with tc.tile_pool(name="big", bufs=1) as big:
            QT = big.tile([P, NDC, S], f32r, tag="QT")
            KT = big.tile([P, NDC, S], f32r, tag="KT")
            V = big.tile([P, NKT, NHC * VW], f32r, tag="V")
            outT = big.tile([P, NDC, S], f32r, tag="outT")

            # ---- projections ------------------------------------------------
            with (
                tc.tile_pool(name="xc", bufs=16) as xc,
                tc.tile_pool(name="pp", bufs=4, space="PSUM") as pp,
            ):

                def proj_T(x_view, w_dram, out_t):
                    w_t = big.tile([P, DK, DH], f32r, tag="w")
                    nc.sync.dma_start(
                        w_t[:], w_dram.rearrange("(c p) m -> p c m", p=P)
                    )
                    for qb in range(NQB):
                        xts = []
                        for dk in range(DK):
                            xt = xc.tile([P, QB], f32r, tag="xc")
                            nc.sync.dma_start(
                                xt[:], x_view[:, dk, qb * QB : (qb + 1) * QB]
                            )
                            xts.append(xt)
                        for dc in range(NDC):
                            pt = pp.tile([P, QB], f32, tag="pp")
                            for dk in range(DK):
                                nc.tensor.matmul(
                                    pt[:],
                                    w_t[:, dk, dc * P : (dc + 1) * P],
                                    xts[dk][:],
                                    start=(dk == 0),
                                    stop=(dk == DK - 1),
                                )
                            nc.vector.tensor_copy(
                                out_t[:, dc, qb * QB : (qb + 1) * QB], pt[:]
                            )

                proj_T(xq_v, wq, QT)
                proj_T(xk_v, wk, KT)

                # V projection (tokens-on-partitions) + ones columns
                nc.vector.memset(V[:].bitcast(f32), 1.0)
                wv_t = big.tile([P, DK, DH], f32r, tag="w")
                nc.sync.dma_start(wv_t[:], wv.rearrange("(c p) m -> p c m", p=P))
                for qb in range(NQB):
                    xts = []
                    for dk in range(DK):
                        xt = xc.tile([P, QB], f32r, tag="xc")
                        nc.sync.dma_start(
                            xt[:], xv_v[:, dk, qb * QB : (qb + 1) * QB]
                        )
                        xts.append(xt)
                    for kt_in in range(QB // P):
                        kt = qb * (QB // P) + kt_in
                        pt = pp.tile([P, DH], f32, tag="pp")
                        for dk in range(DK):
                            nc.tensor.matmul(
                                pt[:],
                                xts[dk][:, kt_in * P : (kt_in + 1) * P],
                                wv_t[:, dk, :],
                                start=(dk == 0),
                                stop=(dk == DK - 1),
                            )
                        nc.vector.tensor_copy(
                            V[:, kt].rearrange("p (h c) -> p h c", c=VW)[
                                :, :, 0:HD
                            ],
                            pt[:].rearrange("p (h c) -> p h c", c=HD),
                        )

            # wo loads while attention runs (slot frees when wv_t is done)
            wo_t = big.tile([P, NDC, D], f32r, tag="w")
            nc.sync.dma_start(wo_t[:], wo.rearrange("(c p) m -> p c m", p=P))

            # ---- attention --------------------------------------------------
            # Software pipeline per head: scores(kt+1) is emitted BEFORE
            # attn@V(kt) so the PE streams through exp's latency, and sc uses
            # two 2-bank psum tiles so the slot frees as soon as its exp ran.
            with (
                tc.tile_pool(name="attn", bufs=2) as attnp,
                tc.tile_pool(name="small", bufs=1) as small,
                tc.tile_pool(name="acc", bufs=1, space="PSUM") as accp,
                tc.tile_pool(name="sc", bufs=2, space="PSUM") as scp,
            ):
                for h in range(NHC):
                    hp, hc = h % 2, h // 2
                    r0 = 64 * hp

                    def emit_scores(kt, hc=hc, r0=r0):
                        tiles = []
                        for half in range(2):
                            sct = scp.tile([P, 2 * QB], f32, tag="sc")
                            for j in range(2):
                                qb = half * 2 + j
                                nc.tensor.matmul(
                                    sct[:, j * QB : (j + 1) * QB],
                                    KT[r0 : r0 + 64, hc, kt * P : (kt + 1) * P],
                                    QT[r0 : r0 + 64, hc, qb * QB : (qb + 1) * QB],
                                    start=True,
                                    stop=True,
                                )
                            tiles.append(sct)
                        return tiles

                    acc = accp.tile([P, S], f32, tag="acc")
                    sc_cur = emit_scores(0)
                    for kt in range(NKT):
                        at_t = attnp.tile([P, S], f32r, tag="attn")
                        for half in range(2):
                            nc.scalar.activation(
                                at_t[:, half * 2 * QB : (half + 1) * 2 * QB],
                                sc_cur[half][:],
                                Exp,
                            )
                        if kt + 1 < NKT:
                            sc_cur = emit_scores(kt + 1)
                        for qb in range(NQB):
                            nc.tensor.matmul(
                                acc[0:VW, qb * QB : (qb + 1) * QB],
                                V[:, kt, h * VW : (h + 1) * VW],
                                at_t[:, qb * QB : (qb + 1) * QB],
                                start=(kt == 0),
                                stop=(kt == NKT - 1),
                            )
                    # denominator (acc row 64) -> all 64 partitions
                    bc = small.tile([64, S], f32, tag="bc")
                    nc.vector.stream_shuffle(bc[0:32, :], acc[64:96, :], [0] * 32)
                    nc.vector.stream_shuffle(bc[32:64, :], acc[64:96, :], [0] * 32)
                    rec = small.tile([64, S], f32, tag="rec")
                    nc.vector.reciprocal(rec[:], bc[:])
                    if hp == 0:
                        nc.vector.tensor_mul(
                            outT[0:64, hc, :], acc[0:HD, :], rec[:]
                        )
                    else:
                        tmp = small.tile([64, S], f32r, tag="tmp")
                        nc.vector.tensor_mul(tmp[:], acc[0:HD, :], rec[:])
                        nc.sync.dma_start(outT[64:128, hc, :], tmp[:])

            # ---- output projection -----------------------------------------
            with (
                tc.tile_pool(name="stage", bufs=2) as stage,
                tc.tile_pool(name="po", bufs=2, space="PSUM") as pop,
            ):
                for qt in range(NKT):
                    po = pop.tile([P, D], f32, tag="po")
                    for do in range(2):
                        for dc in range(NDC):
                            nc.tensor.matmul(
                                po[:, do * QB : (do + 1) * QB],
                                outT[:, dc, qt * P : (qt + 1) * P],
                                wo_t[:, dc, do * QB : (do + 1) * QB],
                                start=(dc == 0),
                                stop=(dc == NDC - 1),
                            )
                    st = stage.tile([P, D], f32, tag="st")
                    nc.vector.tensor_copy(st[:], po[:])
                    nc.sync.dma_start(part[qt * P : (qt + 1) * P, :], st[:])

    nc.compile()
    return nc


def _get_prog():
    if _PROG[0] is None:
        _PROG[0] = _build()
    return _PROG[0]


def make_in_maps(query, key, value, Wq, Wk, Wv, Wo):
    scale = np.float32(1.0 / np.sqrt(D))
    Wq_s = (np.asarray(Wq, np.float32) * scale).astype(np.float32)
    Wk_s = np.ascontiguousarray(np.asarray(Wk, np.float32))
    Wv_s = np.ascontiguousarray(np.asarray(Wv, np.float32))
    Wo_s = np.ascontiguousarray(np.asarray(Wo, np.float32))
    in_maps = []
    for b in range(B):
        xqT = np.ascontiguousarray(np.asarray(query[b], np.float32).T)
        xkT = np.ascontiguousarray(np.asarray(key[b], np.float32).T)
        xvT = np.ascontiguousarray(np.asarray(value[b], np.float32).T)
        for hh in range(2):
            sl = slice(hh * DH, (hh + 1) * DH)
            in_maps.append(
                {
                    "xq": xqT,
                    "xk": xkT,
                    "xv": xvT,
                    "wq": np.ascontiguousarray(Wq_s[:, sl]),
                    "wk": np.ascontiguousarray(Wk_s[:, sl]),
                    "wv": np.ascontiguousarray(Wv_s[:, sl]),
                    "wo": np.ascontiguousarray(Wo_s[sl, :]),
                }
            )
    return in_maps


def run(in_maps, trace=False, **kw):
    from concourse.bass_utils import run_bass_kernel_spmd

    nc = _get_prog()
    return run_bass_kernel_spmd(
        nc, in_maps, core_ids=list(range(NCORES)), trace=trace, **kw
    )


def kernel(query, key, value, Wq, Wk, Wv, Wo, bo):
    in_maps = make_in_maps(query, key, value, Wq, Wk, Wv, Wo)
    res = run(in_maps)
    bo = np.asarray(bo, np.float32)
    out = np.empty((B, S, D), np.float32)
    for b in range(B):
        out[b] = res.results[2 * b]["part"] + res.results[2 * b + 1]["part"] + bo
    return out


# revision 5
# speedup vs baseline: 2.5129x; 1.6363x over previous
"""Multi-head attention (B=4, S=2048, D=1024, H=16) on 8 trn2 NeuronCores.

Sharding: batch (4-way) x head-half (2-way).  Core c = 2*b + hh handles batch b
and heads hh*8 .. hh*8+7.  Each core:
  1. QT/KT projections in d-on-partitions layout, V in tokens-on-partitions
     layout with a ones-column per head (softmax denominator comes out of the
     attn@V matmul as row 64).  All matmuls run as float32r (full PE rate,
     fp32-matmul numerics).
  2. Attention is processed per (q-half, head, key-tile), software-pipelined:
     scores_T = K_h @ Q_h^T into a 2-bank PSUM tile, one wide exp on the
     scalar engine -> attn_T slice (streamed flash-style, no SxS
     materialization), attn@V accumulates out_T per head in PSUM; scores for
     kt+1 are emitted before attn@V(kt) so the PE streams through exp latency.
  3. Tail per head: copy out of PSUM (frees banks fast), denominator row
     replicated across partitions with two 32-lane stream shuffles, fast
     Newton reciprocal, multiply into out_T.  The output projection consumes
     out_T directly as lhsT per q-half.  Host sums the two half-head partials
     and adds the bias.
"""

import sys

if "/opt/trn_rl_repo" not in sys.path:
    sys.path.insert(0, "/opt/trn_rl_repo")

import numpy as np

B, S, D = 4, 2048, 1024
H, HD = 16, 64
P = 128
DK = D // P            # 8 contraction chunks for the projections
NKT = S // P           # 16 token tiles
QB = 512
NQB = S // QB          # 4 q blocks
HB = 2 * QB            # q-half width (1024)
DH = 512               # head dims per core (8 heads)
NDC = DH // P          # 4 dout chunks per core
NHC = 8                # heads per core
VW = HD + 1            # V columns per head incl. the ones column
NCORES = 8

_PROG = [None]


def _build():
    import concourse.mybir as mybir
    import concourse.tile as tile
    from concourse import bacc

    f32 = mybir.dt.float32
    f32r = mybir.dt.float32r
    Exp = mybir.ActivationFunctionType.Exp

    nc = bacc.Bacc("TRN2", target_bir_lowering=False, debug=False)
    xq = nc.dram_tensor("xq", [D, S], f32r, kind="ExternalInput").ap()
    xk = nc.dram_tensor("xk", [D, S], f32r, kind="ExternalInput").ap()
    xv = nc.dram_tensor("xv", [D, S], f32r, kind="ExternalInput").ap()
    wq = nc.dram_tensor("wq", [D, DH], f32r, kind="ExternalInput").ap()
    wk = nc.dram_tensor("wk", [D, DH], f32r, kind="ExternalInput").ap()
    wv = nc.dram_tensor("wv", [D, DH], f32r, kind="ExternalInput").ap()
    wo = nc.dram_tensor("wo", [DH, D], f32r, kind="ExternalInput").ap()
    part = nc.dram_tensor("part", [S, D], f32, kind="ExternalOutput").ap()

    xq_v = xq.rearrange("(c p) s -> p c s", p=P)
    xk_v = xk.rearrange("(c p) s -> p c s", p=P)
    xv_v = xv.rearrange("(c p) s -> p c s", p=P)

    with tile.TileContext(nc) as tc:
        with tc.tile_pool(name="big", bufs=1) as big:
            QT = big.tile([P, NDC, S], f32r, tag="QT")
            KT = big.tile([P, NDC, S], f32r, tag="KT")
            V = big.tile([P, NKT, NHC * VW], f32r, tag="V")
            outT = big.tile([P, NDC, S], f32r, tag="outT")

            # ---- projections ------------------------------------------------
            with (
                tc.tile_pool(name="xc", bufs=16) as xc,
                tc.tile_pool(name="pp", bufs=4, space="PSUM") as pp,
            ):

                def proj_T(x_view, w_dram, out_t):
                    w_t = big.tile([P, DK, DH], f32r, tag="w")
                    nc.sync.dma_start(
                        w_t[:], w_dram.rearrange("(c p) m -> p c m", p=P)
                    )
                    for qb in range(NQB):
                        xts = []
                        for dk in range(DK):
                            xt = xc.tile([P, QB], f32r, tag="xc")
                            nc.sync.dma_start(
                                xt[:], x_view[:, dk, qb * QB : (qb + 1) * QB]
                            )
                            xts.append(xt)
                        for dc in range(NDC):
                            pt = pp.tile([P, QB], f32, tag="pp")
                            for dk in range(DK):
                                nc.tensor.matmul(
                                    pt[:],
                                    w_t[:, dk, dc * P : (dc + 1) * P],
                                    xts[dk][:],
                                    start=(dk == 0),
                                    stop=(dk == DK - 1),
                                )
                            dst = out_t[:, dc, qb * QB : (qb + 1) * QB]
                            if dc % 2 == 0:
                                nc.vector.tensor_copy(dst, pt[:])
                            else:
                                nc.scalar.copy(dst, pt[:])

                proj_T(xq_v, wq, QT)
                proj_T(xk_v, wk, KT)

                # V projection (tokens-on-partitions) + ones columns
                nc.vector.memset(V[:].bitcast(f32), 1.0)
                wv_t = big.tile([P, DK, DH], f32r, tag="w")
                nc.sync.dma_start(wv_t[:], wv.rearrange("(c p) m -> p c m", p=P))
                for qb in range(NQB):
                    xts = []
                    for dk in range(DK):
                        xt = xc.tile([P, QB], f32r, tag="xc")
                        nc.sync.dma_start(
                            xt[:], xv_v[:, dk, qb * QB : (qb + 1) * QB]
                        )
                        xts.append(xt)
                    for kt_in in range(QB // P):
                        kt = qb * (QB // P) + kt_in
                        pt = pp.tile([P, DH], f32, tag="pp")
                        for dk in range(DK):
                            nc.tensor.matmul(
                                pt[:],
                                xts[dk][:, kt_in * P : (kt_in + 1) * P],
                                wv_t[:, dk, :],
                                start=(dk == 0),
                                stop=(dk == DK - 1),
                            )
                        nc.vector.tensor_copy(
                            V[:, kt].rearrange("p (h c) -> p h c", c=VW)[
                                :, :, 0:HD
                            ],
                            pt[:].rearrange("p (h c) -> p h c", c=HD),
                        )

            # wo loads while attention runs (slot frees when wv_t is done)
            wo_t = big.tile([P, NDC, D], f32r, tag="w")
            nc.sync.dma_start(wo_t[:], wo.rearrange("(c p) m -> p c m", p=P))

            # ---- attention + output projection, per q-half ------------------
            with (
                tc.tile_pool(name="attn", bufs=3) as attnp,
                tc.tile_pool(name="tail", bufs=2) as tailp,
                tc.tile_pool(name="stage", bufs=2) as stage,
                tc.tile_pool(name="acc", bufs=1, space="PSUM") as accp,
                tc.tile_pool(name="sc", bufs=2, space="PSUM") as scp,
                tc.tile_pool(name="po", bufs=2, space="PSUM") as pop,
            ):
                for half in range(2):
                    c0 = half * HB
                    for h in range(NHC):
                        hp, hc = h % 2, h // 2
                        r0 = 64 * hp

                        def emit_scores(kt, hc=hc, r0=r0, c0=c0):
                            sct = scp.tile([P, HB], f32, tag="sc")
                            for j in range(2):
                                nc.tensor.matmul(
                                    sct[:, j * QB : (j + 1) * QB],
                                    KT[r0 : r0 + 64, hc, kt * P : (kt + 1) * P],
                                    QT[
                                        r0 : r0 + 64,
                                        hc,
                                        c0 + j * QB : c0 + (j + 1) * QB,
                                    ],
                                    start=True,
                                    stop=True,
                                )
                            return sct

                        acc = accp.tile([P, HB], f32, tag="acc")
                        sc_cur = emit_scores(0)
                        for kt in range(NKT):
                            at_t = attnp.tile([P, HB], f32r, tag="attn")
                            nc.scalar.activation(at_t[:], sc_cur[:], Exp)
                            if kt + 1 < NKT:
                                sc_cur = emit_scores(kt + 1)
                            for j in range(2):
                                nc.tensor.matmul(
                                    acc[0:VW, j * QB : (j + 1) * QB],
                                    V[:, kt, h * VW : (h + 1) * VW],
                                    at_t[:, j * QB : (j + 1) * QB],
                                    start=(kt == 0),
                                    stop=(kt == NKT - 1),
                                )
                        # tail: evacuate psum, replicate denominator, divide
                        asb = tailp.tile([96, HB], f32, tag="asb")
                        nc.vector.tensor_copy(asb[0:VW, :], acc[0:VW, :])
                        bc = tailp.tile([64, HB], f32, tag="bc")
                        nc.vector.stream_shuffle(
                            bc[0:32, :], asb[64:96, :], [0] * 32
                        )
                        nc.vector.stream_shuffle(
                            bc[32:64, :], asb[64:96, :], [0] * 32
                        )
                        rec = tailp.tile([64, HB], f32, tag="rec")
                        scr = tailp.tile([64, HB], f32, tag="scr")
                        nc.vector.reciprocal_approx_accurate(
                            rec[:], bc[:], scr[:]
                        )
                        if hp == 0:
                            nc.vector.tensor_mul(
                                outT[0:64, hc, c0 : c0 + HB],
                                asb[0:HD, :],
                                rec[:],
                            )
                        else:
                            tmp = tailp.tile([64, HB], f32r, tag="tmp")
                            nc.vector.tensor_mul(tmp[:], asb[0:HD, :], rec[:])
                            nc.sync.dma_start(
                                outT[64:128, hc, c0 : c0 + HB], tmp[:]
                            )

                    # output projection for this q-half
                    for qt in range(half * (NKT // 2), (half + 1) * (NKT // 2)):
                        for do in range(2):
                            po = pop.tile([P, QB], f32, tag="po")
                            for dc in range(NDC):
                                nc.tensor.matmul(
                                    po[:],
                                    outT[:, dc, qt * P : (qt + 1) * P],
                                    wo_t[:, dc, do * QB : (do + 1) * QB],
                                    start=(dc == 0),
                                    stop=(dc == NDC - 1),
                                )
                            st = stage.tile([P, QB], f32, tag="st")
                            nc.vector.tensor_copy(st[:], po[:])
                            nc.sync.dma_start(
                                part[
                                    qt * P : (qt + 1) * P,
                                    do * QB : (do + 1) * QB,
                                ],
                                st[:],
                            )

    nc.compile()
    return nc


def _get_prog():
    if _PROG[0] is None:
        _PROG[0] = _build()
    return _PROG[0]


def make_in_maps(query, key, value, Wq, Wk, Wv, Wo):
    scale = np.float32(1.0 / np.sqrt(D))
    Wq_s = (np.asarray(Wq, np.float32) * scale).astype(np.float32)
    Wk_s = np.ascontiguousarray(np.asarray(Wk, np.float32))
    Wv_s = np.ascontiguousarray(np.asarray(Wv, np.float32))
    Wo_s = np.ascontiguousarray(np.asarray(Wo, np.float32))
    in_maps = []
    for b in range(B):
        xqT = np.ascontiguousarray(np.asarray(query[b], np.float32).T)
        xkT = np.ascontiguousarray(np.asarray(key[b], np.float32).T)
        xvT = np.ascontiguousarray(np.asarray(value[b], np.float32).T)
        for hh in range(2):
            sl = slice(hh * DH, (hh + 1) * DH)
            in_maps.append(
                {
                    "xq": xqT,
                    "xk": xkT,
                    "xv": xvT,
                    "wq": np.ascontiguousarray(Wq_s[:, sl]),
                    "wk": np.ascontiguousarray(Wk_s[:, sl]),
                    "wv": np.ascontiguousarray(Wv_s[:, sl]),
                    "wo": np.ascontiguousarray(Wo_s[sl, :]),
                }
            )
    return in_maps


def run(in_maps, trace=False, **kw):
    from concourse.bass_utils import run_bass_kernel_spmd

    nc = _get_prog()
    return run_bass_kernel_spmd(
        nc, in_maps, core_ids=list(range(NCORES)), trace=trace, **kw
    )


def kernel(query, key, value, Wq, Wk, Wv, Wo, bo):
    in_maps = make_in_maps(query, key, value, Wq, Wk, Wv, Wo)
    res = run(in_maps)
    bo = np.asarray(bo, np.float32)
    out = np.empty((B, S, D), np.float32)
    for b in range(B):
        out[b] = res.results[2 * b]["part"] + res.results[2 * b + 1]["part"] + bo
    return out
